# revision 4
# baseline (speedup 1.0000x reference)
"""KDE mixture kernel for Trainium2 (8 NeuronCores, SPMD, P-sharded).

Computes, for P=16384 query points and K=65536 weighted Gaussian centers
(bandwidth 0.5, z-axis scaled by 2):
    f(p)      = sum_k w_k exp(-2*||p'-s'_k||^2)
    log_grad  = inv_bw2 * (sum_k w_k exp(.) s'_k - f p') / (f+eps) * scale

Mapping:
  - points sharded across 8 cores (2048 each); centers/weights replicated.
  - mm1 (PE): sq[k,p] = ||s'||^2 + ||p'||^2 - 2 s'.p' via augmented vectors
    A=[s,s2,1], B=[-2p,1,p2], fp16 hi/lo split packed into a 15-deep
    contraction (A_hi.B_hi + A_hi.B_lo + A_lo.B_hi) -> PSUM fp32.
  - ACT: km = exp(-2*sq), PSUM -> SBUF fp16, batched 3 k-tiles (FD=1536).
  - mm2 (PE): [wkS; f] = (w*[s,1]*2^13)^T @ km accumulated in PSUM.
  - host epilogue (O(P)): divide / rescale exactly like the reference.
"""

import sys
import types
import numpy as np

try:
    import concourse.bass as bass
except ImportError:
    sys.path.insert(0, "/opt/trn_rl_repo")
    import concourse.bass as bass

import concourse.mybir as mybir
import concourse.tile as tile
from contextlib import ExitStack
from concourse.bass_utils import run_bass_kernel_spmd

P, K, D = 16384, 65536, 3
N_CORES = 8
P_LOC = P // N_CORES          # 2048 points per core
CHUNK = 512                   # point chunk (one PSUM bank of fp32)
N_CHUNKS = P_LOC // CHUNK     # 4
KT = K // 128                 # 512 k-tiles of 128 centers
GROUP = 3                     # k-tiles per ACT batch (3 PSUM banks)
N_GROUPS = (KT + GROUP - 1) // GROUP  # 171 (170x3 + 1x2)
W_SCALE = 2.0 ** 13           # keeps w*s in fp16 normal range
EPS = 1e-12
BANDWIDTH = 0.5
Z_FACTOR = 2.0
INV_BW2 = 1.0 / (BANDWIDTH * BANDWIDTH)

_PATCHED = False


def _patch_tail_drain():
    """The walrus build here rejects >2 sem waits on one instruction; Tile's
    tail drain aggregates one wait per live semaphore. Split them into
    individual wait instructions."""
    global _PATCHED
    if _PATCHED:
        return
    _PATCHED = True

    def _drain_and_barrier_split(self, tick_clock, wait_clock):
        from concourse.tile import ScopedClock
        probe = self.nc.sync.nop(nofuse=True)
        wait_clock.add_sem_waits(probe.ins, ScopedClock({None: tick_clock.global_clock}))
        si = probe.ins.sync_info
        waits = list(si.on_wait) if si is not None else []
        probe.ins.sync_info = None
        handles = {h.num: h for h in self.sems.allocated().values()}
        for w in waits:
            assert w.wait_mode == "sem-ge-imm", w.wait_mode
            self.nc.sync.wait_ge(handles[w.id], w.wait_value)
        self.nc.sync.drain()
        self.nc.all_engine_barrier()
        popped = self.nc._tile_sem_poison_stack.pop()
        assert popped is self._sem_poison
        self.nc.clear_and_free_semaphores(list(self.sems.allocated().values()))
        self.nc.all_engine_barrier()

    tile.TileContext._drain_and_barrier = _drain_and_barrier_split


def _split_waits(nc):
    """This walrus build accepts at most ONE sem wait per instruction.
    Hoist extra waits onto same-engine NOPs inserted just before."""
    import bass_rust
    for fn in nc.m.functions:
        for bb in fn.blocks:
            lst = bb.instructions
            i = 0
            while i < len(lst):
                ins = lst[i]
                si = ins.sync_info
                if si is not None and len(si.on_wait) > 1:
                    waits = list(si.on_wait)
                    ups = list(si.on_update)
                    carriers = []
                    for w in waits[:-1]:
                        nop = nc.engines[ins.engine].nop(nofuse=True).ins
                        src = nc.cur_bb.bb.instructions
                        assert src[-1] is nop
                        src.pop()
                        nop.sync_info = bass_rust.SyncInfo(on_wait=[w], on_update=[])
                        carriers.append(nop)
                    ins.sync_info = bass_rust.SyncInfo(
                        on_wait=[waits[-1]], on_update=ups)
                    for k, nopi in enumerate(carriers):
                        lst.insert(i + k, nopi)
                    i += len(carriers)
                i += 1


def _build_program():
    _patch_tail_drain()
    nc = bass.Bass("TRN2", debug=False, target_bir_lowering=False)
    f16, f32 = mybir.dt.float16, mybir.dt.float32

    lhs1_d = nc.dram_tensor("lhs1", [15, K], f16, kind="ExternalInput")
    rhs1_d = nc.dram_tensor("rhs1", [15, P_LOC], f16, kind="ExternalInput")
    w4_d = nc.dram_tensor("w4", [128, KT * 4], f16, kind="ExternalInput")
    out_d = nc.dram_tensor("out", [4, P_LOC], f32, kind="ExternalOutput")

    with tile.TileContext(nc) as tc, ExitStack() as ctx:
        const_pool = ctx.enter_context(tc.tile_pool(name="const", bufs=1))
        sq_pool = ctx.enter_context(tc.tile_pool(name="sq", bufs=2, space="PSUM"))
        acc_pool = ctx.enter_context(tc.tile_pool(name="acc", bufs=2, space="PSUM"))
        km_pool = ctx.enter_context(tc.tile_pool(name="km", bufs=3))
        out_pool = ctx.enter_context(tc.tile_pool(name="outs", bufs=2))

        lhs1_sb = const_pool.tile([15, K], f16, tag="lhs1")
        nc.gpsimd.dma_start(lhs1_sb[:], lhs1_d.ap()[:])
        rhs1_sb = const_pool.tile([15, P_LOC], f16, tag="rhs1")
        nc.gpsimd.dma_start(rhs1_sb[:], rhs1_d.ap()[:])
        w4_sb = const_pool.tile([128, KT * 4], f16, tag="w4")
        nc.gpsimd.dma_start(w4_sb[:], w4_d.ap()[:])

        Exp = mybir.ActivationFunctionType.Exp
        acc_tiles = {}

        def emit_mm2(c, g, km, kts):
            if g == 0:
                acc_tiles[c] = acc_pool.tile([4, CHUNK], f32, tag="acc", name=f"acc{c}")
            acc = acc_tiles[c]
            for j, kt in enumerate(kts):
                nc.tensor.matmul(
                    acc[:],
                    lhsT=w4_sb[:, kt * 4:(kt + 1) * 4],
                    rhs=km[:, CHUNK * j:CHUNK * (j + 1)],
                    start=(g == 0 and j == 0),
                    stop=(g == N_GROUPS - 1 and j == len(kts) - 1),
                )
            if g == N_GROUPS - 1:
                ot = out_pool.tile([4, CHUNK], f32, tag="ot", name=f"ot{c}")
                nc.vector.tensor_copy(ot[:], acc[:])
                nc.gpsimd.dma_start(out_d.ap()[:, c * CHUNK:(c + 1) * CHUNK], ot[:])

        pending = None
        for c in range(N_CHUNKS):
            for g in range(N_GROUPS):
                kts = list(range(g * GROUP, min((g + 1) * GROUP, KT)))
                n = len(kts)
                sq = sq_pool.tile([128, CHUNK * GROUP], f32, tag="sq", name=f"sq{c}_{g}")
                for j, kt in enumerate(kts):
                    nc.tensor.matmul(
                        sq[:, CHUNK * j:CHUNK * (j + 1)],
                        lhsT=lhs1_sb[:, kt * 128:(kt + 1) * 128],
                        rhs=rhs1_sb[:, c * CHUNK:(c + 1) * CHUNK],
                        start=True,
                        stop=True,
                    )
                km = km_pool.tile([128, CHUNK * GROUP], f16, tag="km", name=f"km{c}_{g}")
                nc.scalar.activation(
                    km[:, :CHUNK * n], sq[:, :CHUNK * n], Exp, scale=-2.0
                )
                if pending is not None:
                    emit_mm2(*pending)
                pending = (c, g, km, kts)
        emit_mm2(*pending)

    _split_waits(nc)
    return nc


_NC_CACHE = None


def _get_program():
    global _NC_CACHE
    if _NC_CACHE is None:
        _NC_CACHE = _build_program()
    return _NC_CACHE


def _preprocess(points, spatial_data, weights):
    scale = np.array([1.0, 1.0, Z_FACTOR], dtype=np.float32)
    p = points.astype(np.float32) * scale
    s = spatial_data.astype(np.float32) * scale
    w = weights.astype(np.float64)
    w = w / (w.sum() + EPS)

    ones_k = np.ones(K, np.float32)
    A = np.stack([s[:, 0], s[:, 1], s[:, 2], (s * s).sum(-1), ones_k], 0)
    B = np.stack(
        [-2.0 * p[:, 0], -2.0 * p[:, 1], -2.0 * p[:, 2],
         np.ones(P, np.float32), (p * p).sum(-1)], 0)
    A_hi = A.astype(np.float16)
    A_lo = (A - A_hi.astype(np.float32)).astype(np.float16)
    B_hi = B.astype(np.float16)
    B_lo = (B - B_hi.astype(np.float32)).astype(np.float16)
    lhs1 = np.ascontiguousarray(np.concatenate([A_hi, A_hi, A_lo], 0))
    rhs1 = np.ascontiguousarray(np.concatenate([B_hi, B_lo, B_hi], 0))

    W4 = np.empty((K, 4), np.float64)
    W4[:, :3] = w[:, None] * s.astype(np.float64)
    W4[:, 3] = w
    W4 *= W_SCALE
    w4 = np.ascontiguousarray(
        W4.astype(np.float16).reshape(KT, 128, 4).transpose(1, 0, 2).reshape(128, KT * 4))
    return p, w, lhs1, rhs1, w4


def run(points, spatial_data, weights, trace=False):
    assert points.shape == (P, D) and spatial_data.shape == (K, D)
    p, w, lhs1, rhs1, w4 = _preprocess(points, spatial_data, weights)

    in_maps = []
    for c in range(N_CORES):
        in_maps.append({
            "lhs1": lhs1,
            "rhs1": np.ascontiguousarray(rhs1[:, c * P_LOC:(c + 1) * P_LOC]),
            "w4": w4,
        })

    nc = _get_program()
    res = run_bass_kernel_spmd(
        nc, in_maps, core_ids=list(range(N_CORES)), trace=trace)

    out = np.concatenate([res.results[c]["out"] for c in range(N_CORES)], axis=1)
    wkS = out[:3].astype(np.float64).T / W_SCALE      # [P, 3]
    f = out[3].astype(np.float64) / W_SCALE           # [P]

    p64 = p.astype(np.float64)
    scale = np.array([1.0, 1.0, Z_FACTOR], dtype=np.float64)
    log_grad = INV_BW2 * (wkS - f[:, None] * p64) / (f[:, None] + EPS) * scale
    return (f.astype(np.float32), log_grad.astype(np.float32)), res


def kernel(points, spatial_data, weights):
    (fvals, log_grad), _ = run(points, spatial_data, weights, trace=False)
    return fvals, log_grad


# revision 11
# speedup vs baseline: 1.6230x; 1.6230x over previous
"""KDE mixture kernel for Trainium2 (8 NeuronCores, SPMD, P-sharded).

Computes, for P=16384 query points and K=65536 weighted Gaussian centers
(bandwidth 0.5, z-axis scaled by 2):
    f(p)      = sum_k w_k exp(-2*||p'-s'_k||^2)
    log_grad  = inv_bw2 * (sum_k w_k exp(.) s'_k - f p') / (f+eps) * scale

Mapping:
  - points sharded across 8 cores (2048 each); centers/weights replicated.
  - mm1 (PE): sq[k,p] = ||s'||^2 + ||p'||^2 - 2 s'.p' via augmented vectors
    A=[s,s2,1], B=[-2p,1,p2], fp16 hi/lo split packed into a 15-deep
    contraction (A_hi.B_hi + A_hi.B_lo + A_lo.B_hi) -> PSUM fp32.
  - ACT: km = exp(-2*sq), PSUM -> SBUF fp16, batched 3 k-tiles (FD=1536).
  - mm2 (PE): [wkS; f] = (w*[s,1]*2^13)^T @ km accumulated in PSUM.
  - host epilogue (O(P)): divide / rescale exactly like the reference.
"""

import sys
import types
import numpy as np

try:
    import concourse.bass as bass
except ImportError:
    sys.path.insert(0, "/opt/trn_rl_repo")
    import concourse.bass as bass

import concourse.mybir as mybir
import concourse.tile as tile
from contextlib import ExitStack
from concourse.bass_utils import run_bass_kernel_spmd

P, K, D = 16384, 65536, 3
N_CORES = 8
P_LOC = P // N_CORES          # 2048 points per core
CHUNK = 512                   # point chunk (one PSUM bank of fp32)
N_CHUNKS = P_LOC // CHUNK     # 4
KT = K // 128                 # 512 k-tiles of 128 centers
GROUP = 3                     # k-tiles per ACT batch (3 PSUM banks)
N_GROUPS = (KT + GROUP - 1) // GROUP  # 171 (170x3 + 1x2)
W_SCALE = 2.0 ** 13           # keeps w*s in fp16 normal range
EPS = 1e-12
BANDWIDTH = 0.5
Z_FACTOR = 2.0
INV_BW2 = 1.0 / (BANDWIDTH * BANDWIDTH)

_PATCHED = False


def _patch_tail_drain():
    """The walrus build here rejects >2 sem waits on one instruction; Tile's
    tail drain aggregates one wait per live semaphore. Split them into
    individual wait instructions."""
    global _PATCHED
    if _PATCHED:
        return
    _PATCHED = True

    def _drain_and_barrier_split(self, tick_clock, wait_clock):
        from concourse.tile import ScopedClock
        probe = self.nc.sync.nop(nofuse=True)
        wait_clock.add_sem_waits(probe.ins, ScopedClock({None: tick_clock.global_clock}))
        si = probe.ins.sync_info
        waits = list(si.on_wait) if si is not None else []
        probe.ins.sync_info = None
        handles = {h.num: h for h in self.sems.allocated().values()}
        for w in waits:
            assert w.wait_mode == "sem-ge-imm", w.wait_mode
            self.nc.sync.wait_ge(handles[w.id], w.wait_value)
        self.nc.sync.drain()
        self.nc.all_engine_barrier()
        popped = self.nc._tile_sem_poison_stack.pop()
        assert popped is self._sem_poison
        self.nc.clear_and_free_semaphores(list(self.sems.allocated().values()))
        self.nc.all_engine_barrier()

    tile.TileContext._drain_and_barrier = _drain_and_barrier_split


def _split_waits(nc):
    """This walrus build accepts at most ONE sem wait per instruction.
    Hoist extra waits onto same-engine NOPs inserted just before."""
    import bass_rust
    for fn in nc.m.functions:
        for bb in fn.blocks:
            lst = bb.instructions
            i = 0
            while i < len(lst):
                ins = lst[i]
                si = ins.sync_info
                if si is not None and len(si.on_wait) > 1:
                    waits = list(si.on_wait)
                    ups = list(si.on_update)
                    carriers = []
                    for w in waits[:-1]:
                        nop = nc.engines[ins.engine].nop(nofuse=True).ins
                        src = nc.cur_bb.bb.instructions
                        assert src[-1] is nop
                        src.pop()
                        nop.sync_info = bass_rust.SyncInfo(on_wait=[w], on_update=[])
                        carriers.append(nop)
                    ins.sync_info = bass_rust.SyncInfo(
                        on_wait=[waits[-1]], on_update=ups)
                    for k, nopi in enumerate(carriers):
                        lst.insert(i + k, nopi)
                    i += len(carriers)
                i += 1


def _build_program():
    _patch_tail_drain()
    nc = bass.Bass("TRN2", debug=False, target_bir_lowering=False)
    f16, f32 = mybir.dt.float16, mybir.dt.float32

    lhs1_d = nc.dram_tensor("lhs1", [15, K], f16, kind="ExternalInput")
    rhs1_d = nc.dram_tensor("rhs1", [128, P_LOC], f16, kind="ExternalInput")
    w4_d = nc.dram_tensor("w4", [128, KT * 8], f16, kind="ExternalInput")
    out_d = nc.dram_tensor("out", [8, P_LOC], f32, kind="ExternalOutput")

    with tile.TileContext(nc) as tc, ExitStack() as ctx:
        const_pool = ctx.enter_context(tc.tile_pool(name="const", bufs=1))
        sq_pool = ctx.enter_context(tc.tile_pool(name="sq", bufs=2, space="PSUM"))
        acc_pool = ctx.enter_context(tc.tile_pool(name="acc", bufs=2, space="PSUM"))
        km_pool = ctx.enter_context(tc.tile_pool(name="km", bufs=3))
        out_pool = ctx.enter_context(tc.tile_pool(name="outs", bufs=2))

        # Contraction padded to 128 so FWL kicks in (KC=15 matmuls measured
        # 1.8x slower). Rows 15-127 of lhs1 are zeroed; rhs1 rows 15-127 are
        # zeros from the host.
        lhs1_sb = const_pool.tile([128, K], f16, tag="lhs1")
        # memset free dim is a 16-bit ISA field (<=65535): split in halves
        nc.gpsimd.memset(lhs1_sb[:, :K // 2], 0.0)
        nc.gpsimd.memset(lhs1_sb[:, K // 2:], 0.0)
        nc.gpsimd.dma_start(lhs1_sb[0:15, :], lhs1_d.ap()[:])
        rhs1_sb = const_pool.tile([128, P_LOC], f16, tag="rhs1")
        nc.gpsimd.dma_start(rhs1_sb[:], rhs1_d.ap()[:])
        w4_sb = const_pool.tile([128, KT * 8], f16, tag="w4")
        nc.gpsimd.dma_start(w4_sb[:], w4_d.ap()[:])

        Exp = mybir.ActivationFunctionType.Exp
        acc_tiles = {}

        def emit_mm2(c, g, km, kts):
            if g == 0:
                acc_tiles[c] = acc_pool.tile([8, CHUNK], f32, tag="acc", name=f"acc{c}")
            acc = acc_tiles[c]
            for j, kt in enumerate(kts):
                nc.tensor.matmul(
                    acc[:],
                    lhsT=w4_sb[:, kt * 8:(kt + 1) * 8],
                    rhs=km[:, CHUNK * j:CHUNK * (j + 1)],
                    start=(g == 0 and j == 0),
                    stop=(g == N_GROUPS - 1 and j == len(kts) - 1),
                )
            if g == N_GROUPS - 1:
                ot = out_pool.tile([8, CHUNK], f32, tag="ot", name=f"ot{c}")
                nc.vector.tensor_copy(ot[:], acc[:])
                nc.gpsimd.dma_start(out_d.ap()[:, c * CHUNK:(c + 1) * CHUNK], ot[:])

        pending = None
        for c in range(N_CHUNKS):
            for g in range(N_GROUPS):
                kts = list(range(g * GROUP, min((g + 1) * GROUP, KT)))
                n = len(kts)
                sq = sq_pool.tile([128, CHUNK * GROUP], f32, tag="sq", name=f"sq{c}_{g}")
                for j, kt in enumerate(kts):
                    nc.tensor.matmul(
                        sq[:, CHUNK * j:CHUNK * (j + 1)],
                        lhsT=lhs1_sb[:, kt * 128:(kt + 1) * 128],
                        rhs=rhs1_sb[:, c * CHUNK:(c + 1) * CHUNK],
                        start=True,
                        stop=True,
                    )
                km = km_pool.tile([128, CHUNK * GROUP], f16, tag="km", name=f"km{c}_{g}")
                nc.scalar.activation(
                    km[:, :CHUNK * n], sq[:, :CHUNK * n], Exp, scale=-2.0
                )
                if pending is not None:
                    emit_mm2(*pending)
                pending = (c, g, km, kts)
        emit_mm2(*pending)

    _split_waits(nc)
    return nc


_NC_CACHE = None


def _get_program():
    global _NC_CACHE
    if _NC_CACHE is None:
        _NC_CACHE = _build_program()
    return _NC_CACHE


def _preprocess(points, spatial_data, weights):
    scale = np.array([1.0, 1.0, Z_FACTOR], dtype=np.float32)
    p = points.astype(np.float32) * scale
    s = spatial_data.astype(np.float32) * scale
    w = weights.astype(np.float64)
    w = w / (w.sum() + EPS)

    ones_k = np.ones(K, np.float32)
    A = np.stack([s[:, 0], s[:, 1], s[:, 2], (s * s).sum(-1), ones_k], 0)
    B = np.stack(
        [-2.0 * p[:, 0], -2.0 * p[:, 1], -2.0 * p[:, 2],
         np.ones(P, np.float32), (p * p).sum(-1)], 0)
    A_hi = A.astype(np.float16)
    A_lo = (A - A_hi.astype(np.float32)).astype(np.float16)
    B_hi = B.astype(np.float16)
    B_lo = (B - B_hi.astype(np.float32)).astype(np.float16)
    lhs1 = np.ascontiguousarray(np.concatenate([A_hi, A_hi, A_lo], 0))
    rhs1 = np.zeros((128, P), np.float16)
    rhs1[0:15] = np.concatenate([B_hi, B_lo, B_hi], 0)

    W4 = np.empty((K, 4), np.float64)
    W4[:, :3] = w[:, None] * s.astype(np.float64)
    W4[:, 3] = w
    W4 *= W_SCALE
    W4_hi = W4.astype(np.float16)
    W4_lo = (W4 - W4_hi.astype(np.float64)).astype(np.float16)
    W8 = np.concatenate([W4_hi, W4_lo], axis=1)  # [K, 8]
    w4 = np.ascontiguousarray(
        W8.reshape(KT, 128, 8).transpose(1, 0, 2).reshape(128, KT * 8))
    return p, w, lhs1, rhs1, w4


def run(points, spatial_data, weights, trace=False):
    assert points.shape == (P, D) and spatial_data.shape == (K, D)
    p, w, lhs1, rhs1, w4 = _preprocess(points, spatial_data, weights)

    in_maps = []
    for c in range(N_CORES):
        in_maps.append({
            "lhs1": lhs1,
            "rhs1": np.ascontiguousarray(rhs1[:, c * P_LOC:(c + 1) * P_LOC]),
            "w4": w4,
        })

    nc = _get_program()
    res = run_bass_kernel_spmd(
        nc, in_maps, core_ids=list(range(N_CORES)), trace=trace)

    out = np.concatenate([res.results[c]["out"] for c in range(N_CORES)], axis=1)
    out = out.astype(np.float64)
    hi, lo = out[0:4], out[4:8]
    wkS = (hi[:3] + lo[:3]).T / W_SCALE               # [P, 3]
    f = (hi[3] + lo[3]) / W_SCALE                     # [P]

    p64 = p.astype(np.float64)
    scale = np.array([1.0, 1.0, Z_FACTOR], dtype=np.float64)
    log_grad = INV_BW2 * (wkS - f[:, None] * p64) / (f[:, None] + EPS) * scale
    return (f.astype(np.float32), log_grad.astype(np.float32)), res


def kernel(points, spatial_data, weights):
    (fvals, log_grad), _ = run(points, spatial_data, weights, trace=False)
    return fvals, log_grad
